# revision 33
# baseline (speedup 1.0000x reference)
"""Multi-head attention (B=2, T=2048, C=2048, H=16, causal, rotary) on 8
Trainium2 NeuronCores.

Sharding: tensor-parallel over heads x data-parallel over batch.
Core c handles batch b = c // 4 and heads [4*(c%4), 4*(c%4)+4).
Each core computes a partial output y_c = attn_out(4 heads) @ wo_rows;
the host sums the 4 partials per batch (row-parallel wo).

v5 design. Measured facts this build encodes: back-to-back matmuls into
the SAME psum bank pipeline at ~1 col/cycle; every psum-bank switch
costs ~45ns; a start+stop matmul's visible latency includes a ~173ns
pipeline drain. So long same-bank accumulation runs are kept intact
(fine-grained interleaving of independent work into attention made
everything slower in v4), and attention cuts its matmul count instead:
  - rowsums for full (sub-diagonal) quads of key-blocks are pre-summed
    on DVE (bf16, 2x mode) and hit the PE as ONE ones-stationary matmul
    per quad; only the 4 diagonal blocks keep per-block rowsums.
  - AV (and diagonal rowsum) matmuls are emitted in adjacent pairs so
    every second one continues a same-bank accumulation chain.
  - wo(t-1) + proj(t+1) are emitted per head after attention(t,h)
    (head-granularity interleave).
  - rope: 3 DVE muls (partition-swapped sin products; PSUM operand is
    exempt from the equal-base-partition rule) + sub/add alternating
    DVE/GpSimd. t0 ropes run in (q_h,k_h) pairs so head h's tables are
    ready when attention(0,h) arrives.
  - x staged [NT*C, TCH] and y [4*T, 512]: all big DMAs contiguous.
  - psum banks: pp(proj+wo+norm bcast)=2, pss(scores)=3, pso(AV)=2,
    psr(rowsum)=1.
"""

import math
import os
import sys
from contextlib import ExitStack

import numpy as np

for _p in ("/opt/trn_rl_repo", "/root/.axon_site/_ro/trn_rl_repo"):
    if os.path.isdir(_p) and _p not in sys.path:
        sys.path.append(_p)

import bass_rust
import ml_dtypes
import concourse.bass as bass
import concourse.mybir as mybir
import concourse.tile as tile
from concourse import library_config
from concourse.bass_utils import run_bass_kernel_spmd
from concourse.vector_clock import ScopedClock, VectorClock

B, T, C, H = 2, 2048, 2048, 16
D = 128
HPC = H // 4          # 4 heads per core
DH = HPC * D          # 512 head-dims per core
NCH = C // 128        # 16 contraction chunks
TCH = 512             # t-chunk == q-chunk
NT = T // TCH         # 4
N_CORES = 8
SCALE = 1.0 / math.sqrt(D)

f32 = mybir.dt.float32
f32r = mybir.dt.float32r
bf16 = mybir.dt.bfloat16
AF = mybir.ActivationFunctionType
BF16NP = ml_dtypes.bfloat16


# --------------------------------------------------------------------------
# toolchain workarounds (from the known-good baseline)
# --------------------------------------------------------------------------
def _patched_drain_and_barrier(self, tick_clock, wait_clock):
    """walrus codegen accepts only one sem wait on an InstDrain; emit one
    drain per outstanding proc instead of one drain with N waits."""
    ticks = list(tick_clock.global_clock)
    for i, t in enumerate(ticks):
        if t <= 0:
            continue
        sub = VectorClock([t if j == i else 0 for j in range(len(ticks))])
        d = self.nc.sync.drain()
        wait_clock.add_sem_waits(d.ins, ScopedClock({None: sub}))
    self.nc.all_engine_barrier()
    assert self.sems is not None
    popped = self.nc._tile_sem_poison_stack.pop()
    assert popped is self._sem_poison
    self.nc.clear_and_free_semaphores(list(self.sems.allocated().values()))
    self.nc.all_engine_barrier()


tile.TileContext._drain_and_barrier = _patched_drain_and_barrier

_SPLIT_OPS = {
    "Matmult", "Drain", "DMACopy", "DMATransposeAnt", "Activation", "TensorTensor", "TensorReduce",
    "TensorCopy", "Reciprocal", "TensorScalarPtr", "TensorScalar",
    "CopyPredicated", "Memset", "NoOp", "Pool", "Max", "MaxIndex",
    "StreamShuffle", "StreamTranspose", "TensorTensorScan",
    "ScalarTensorTensor", "TensorTensorReduce", "Iota", "BNStats",
    "BNStatsAggregate", "Select", "PartitionBroadcast",
}
_ws_counter = [0]


def _split_waits(nc, limit=1):
    """walrus encodes a limited number of sem waits on engine instructions
    (fused bf16 LDW+MM and Drain take only one). Move excess waits onto
    same-engine NoOps inserted immediately before; engine program order
    preserves semantics."""
    for f in nc.m.functions:
        for b in f.blocks:
            insts = b.instructions
            i = 0
            while i < len(insts):
                inst = insts[i]
                si = inst.sync_info
                if (
                    inst.opcode not in _SPLIT_OPS
                    or si is None
                    or not si.on_wait
                    or len(si.on_wait) <= limit
                ):
                    i += 1
                    continue
                waits = list(si.on_wait)
                extra, keep = waits[:-limit], waits[-limit:]
                for w in extra:
                    _ws_counter[0] += 1
                    nop = bass_rust.InstNoOp(
                        name=f"I-waitsplit-{_ws_counter[0]}", engine=inst.engine
                    )
                    nop.sync_info = mybir.SyncInfo(on_wait=[w], on_update=[])
                    insts.insert(i, nop)
                    i += 1
                inst.sync_info = mybir.SyncInfo(
                    on_wait=keep,
                    on_update=list(si.on_update) if si.on_update else [],
                )
                i += 1


# --------------------------------------------------------------------------
# kernel build
# --------------------------------------------------------------------------
def _build_nc():
    nc = bass.Bass("TRN2", debug=False, target_bir_lowering=False)

    # x staged as NT row-blocks of [C, TCH] so each (t, ci) chunk is one
    # contiguous 128KB region; y as 4 column-blocks of [T, 512] likewise.
    xT = nc.dram_tensor("xT", [NT * C, TCH], bf16, kind="ExternalInput").ap()
    wq = nc.dram_tensor("wq", [C, DH], bf16, kind="ExternalInput").ap()
    wk = nc.dram_tensor("wk", [C, DH], bf16, kind="ExternalInput").ap()
    wv = nc.dram_tensor("wv", [C, DH], bf16, kind="ExternalInput").ap()
    wo = nc.dram_tensor("wo", [DH, C], bf16, kind="ExternalInput").ap()
    cos2 = nc.dram_tensor("cos2", [128, T], bf16, kind="ExternalInput").ap()
    sin2 = nc.dram_tensor("sin2", [128, T], bf16, kind="ExternalInput").ap()
    mbd = nc.dram_tensor("mb", [128, 128], bf16, kind="ExternalInput").ap()
    y = nc.dram_tensor("y", [4 * T, 512], bf16, kind="ExternalOutput").ap()

    with tile.TileContext(nc) as tc, ExitStack() as es:
        # ---- pools (whole-kernel lifetime) ----
        wpool = es.enter_context(tc.tile_pool(name="w", bufs=1))
        wopool = es.enter_context(tc.tile_pool(name="wo", bufs=1))
        xpool = es.enter_context(tc.tile_pool(name="x", bufs=3))
        cpool = es.enter_context(tc.tile_pool(name="cs", bufs=1))
        persist = es.enter_context(tc.tile_pool(name="qkv", bufs=1))
        rt = es.enter_context(tc.tile_pool(name="rt", bufs=2))
        ep = es.enter_context(tc.tile_pool(name="e", bufs=8))
        sqp = es.enter_context(tc.tile_pool(name="sq", bufs=2))
        otp = es.enter_context(tc.tile_pool(name="ot", bufs=2))
        rp = es.enter_context(tc.tile_pool(name="r", bufs=2))
        ysbp = es.enter_context(tc.tile_pool(name="ysb", bufs=4))

        pp = es.enter_context(tc.tile_pool(name="pp", bufs=2, space="PSUM"))
        pss = es.enter_context(tc.tile_pool(name="pss", bufs=3, space="PSUM"))
        pso = es.enter_context(tc.tile_pool(name="pso", bufs=2, space="PSUM"))
        psr = es.enter_context(tc.tile_pool(name="psr", bufs=1, space="PSUM"))

        # ---- persistent SBUF tensors ----
        # matmul operands are kept in [128, 512] tiles (1KB per-partition
        # rows): operands sliced from wider tiles measurably slow the PE
        # (~216ns vs ~259ns per 512-col matmul).
        qT = {(h, t): persist.tile([128, TCH], bf16, tag=f"qT{h}_{t}", name=f"qT{h}_{t}")
              for h in range(HPC) for t in range(NT)}
        kT = {(h, t): persist.tile([128, TCH], bf16, tag=f"kT{h}_{t}", name=f"kT{h}_{t}")
              for h in range(HPC) for t in range(NT)}
        # vb[kb][:, h*128:(h+1)*128] = V rows of key-block kb for head h
        vb = {kb: persist.tile([128, DH], bf16, tag=f"vb{kb}", name=f"vb{kb}")
              for kb in range(NT * 4)}

        w_tiles = {}
        cs2_t = cpool.tile([128, T], bf16, tag="cos")
        sn2_t = cpool.tile([128, T], bf16, tag="sin")
        mb_t = cpool.tile([128, 128], bf16, tag="mb")
        # all-ones views carved out of the mask tile (mb[:,127]==1 for all k,
        # mb[0,:]==1 for all q) -- saves two DMAs on the startup queue
        onesk = mb_t[:, 127:128]
        ones1 = mb_t[0:1, :]
        wo_tiles = {}

        ot_tiles = {}
        pending = [None]
        rope_par = [0]

        def _emit_norm(h, pso_t, psr_t):
            # 1/rowsum as exp(-ln(r)) on ACT over the tiny [1,512] rowsum
            # vector (DVE reciprocal() is a 3.2us multi-pass op and was
            # congesting the DVE queue), then broadcast across partitions
            # with a ones-stationary matmul and one DVE multiply.
            lnr = rp.tile([1, TCH], f32, tag="lnr")
            nc.scalar.activation(lnr[:], psr_t[0:1, :], AF.Ln)
            binv1 = rp.tile([1, TCH], bf16, tag="binv1")
            nc.scalar.activation(binv1[:], lnr[:], AF.Exp, scale=-1.0)
            nc.tensor.matmul(psr_t[:, :], ones1, binv1[:], start=True, stop=True)
            binv = rp.tile([128, TCH], bf16, tag="binv")
            nc.vector.tensor_copy(binv[:], psr_t[:, :])
            ot = otp.tile([128, TCH], bf16, tag=f"ot{h}")
            nc.vector.tensor_mul(ot[:], pso_t[:], binv[:])
            ot_tiles[h] = ot

        def _copy_out(dst, src, use_act):
            if use_act:
                nc.scalar.copy(dst, src)
            else:
                nc.vector.tensor_copy(dst, src)

        def _rope(ps, dst_t, tsl):
            # de-interleaved pairs: ps[0:64]=real, ps[64:128]=imag.
            # tA = ps*cos2 = [r*cos; i*cos]; tS = partition-swapped sin
            # products [i*sin; r*sin]. out_r = r*cos - i*sin,
            # out_i = r*sin + i*cos.
            tA = rt.tile([128, TCH], f32, tag="rA")
            nc.vector.tensor_mul(tA[:], ps[:], cs2_t[:, tsl])
            tS = rt.tile([128, TCH], f32, tag="rB")
            nc.vector.tensor_mul(tS[0:64, :], ps[64:128, :], sn2_t[0:64, tsl])
            nc.vector.tensor_mul(tS[64:128, :], ps[0:64, :], sn2_t[64:128, tsl])
            eng = nc.vector if rope_par[0] % 2 == 0 else nc.gpsimd
            rope_par[0] += 1
            eng.tensor_sub(dst_t[0:64, :], tA[0:64, :], tS[0:64, :])
            eng.tensor_add(dst_t[64:128, :], tS[64:128, :], tA[64:128, :])

        def _emit_wo_group(qc, qs, src_ot, alt_copies=False):
            # one q-subblock of the deferred output projection; interleaved
            # after each attention head
            row0 = qc * TCH + qs * 128
            for cc in range(C // 512):
                psy = pp.tile([128, 512], f32, tag="pp")
                for hh in range(HPC):
                    nc.tensor.matmul(
                        psy[:],
                        src_ot[hh][:, qs * 128:(qs + 1) * 128],
                        wo_tiles[(hh, cc)][:],
                        start=(hh == 0),
                        stop=(hh == HPC - 1),
                    )
                ysb_c = ysbp.tile([128, 512], bf16, tag="ysb")
                _copy_out(ysb_c[:], psy[:], alt_copies and cc % 2 == 1)
                nc.sync.dma_start(
                    y[cc * T + row0:cc * T + row0 + 128, :], ysb_c[:],
                )

        def _emit_one_proj(t, h, wname, dst, xt):
            tsl_ = bass.ts(t, TCH)
            ps = pp.tile([128, TCH], f32, tag="pp")
            for ci in range(NCH):
                nc.tensor.matmul(
                    ps[:],
                    w_tiles[(wname, ci)][:, h * 128:(h + 1) * 128],
                    xt[ci][:],
                    start=(ci == 0),
                    stop=(ci == NCH - 1),
                )
            _rope(ps, dst[(h, t)], tsl_)

        def _emit_qk_head_proj(t, h, xt):
            _emit_one_proj(t, h, "wq", qT, xt)
            _emit_one_proj(t, h, "wk", kT, xt)

        def _emit_v_tsi(t, tsi, xt, use_act=True):
            ps = pp.tile([128, DH], f32, tag="pp")
            for ci in range(NCH):
                nc.tensor.matmul(
                    ps[:],
                    xt[ci][:, tsi * 128:(tsi + 1) * 128],
                    w_tiles[("wv", ci)][:],
                    start=(ci == 0),
                    stop=(ci == NCH - 1),
                )
            kb = t * 4 + tsi
            _copy_out(vb[kb][:], ps[:], use_act)

        def _emit_qk_proj_t0(xt):
            # run 7 projection groups concurrently across the (still idle)
            # attention psum pools so the PE keeps pace with the w/x DMA
            # stream. Groups are ordered (q0,k0),(q1,k1),... and roped in
            # that order so head h's tables are ready when attention(0,h)
            # arrives. K-h3 is emitted interleaved with the wv-paced
            # V(0)-tsi0 group.
            tsl_ = bass.ts(0, TCH)
            _gp = [("wq", 0, pp, "pp"), ("wk", 0, pso, "o"),
                   ("wq", 1, pss, "s"), ("wk", 1, pso, "o"),
                   ("wq", 2, pss, "s"), ("wk", 2, pp, "pp"),
                   ("wq", 3, pss, "s")]
            groups = [
                (wn, h_, pool.tile([128, TCH], f32, tag=tg, name=f"g0_{wn}{h_}"))
                for wn, h_, pool, tg in _gp
            ]
            for ci in range(NCH):
                for wname, h, ps in groups:
                    nc.tensor.matmul(
                        ps[:],
                        w_tiles[(wname, ci)][:, h * 128:(h + 1) * 128],
                        xt[ci][:],
                        start=(ci == 0),
                        stop=(ci == NCH - 1),
                    )
            for gi in (0, 5, 1, 2, 3, 4, 6):
                wname, h, ps = groups[gi]
                _rope(ps, (qT if wname == "wq" else kT)[(h, 0)], tsl_)

        def _emit_v_proj_t0(xt):
            psk3 = pp.tile([128, TCH], f32, tag="pp", name="psk3")
            for tsi in range(TCH // 128):
                ps = pp.tile([128, DH], f32, tag="pp")
                for ci in range(NCH):
                    nc.tensor.matmul(
                        ps[:],
                        xt[ci][:, tsi * 128:(tsi + 1) * 128],
                        w_tiles[("wv", ci)][:],
                        start=(ci == 0),
                        stop=(ci == NCH - 1),
                    )
                    if tsi == 0:
                        nc.tensor.matmul(
                            psk3[:],
                            w_tiles[("wk", ci)][:, 3 * 128:4 * 128],
                            xt[ci][:],
                            start=(ci == 0), stop=(ci == NCH - 1),
                        )
                if tsi == 0:
                    _rope(psk3, kT[(3, 0)], bass.ts(0, TCH))
                _copy_out(vb[tsi][:], ps[:], True)

        def _prefetch_x(t):
            tiles = []
            for ci in range(NCH):
                x_ = xpool.tile([128, TCH], bf16, tag=f"x{ci}")
                eng = nc.scalar if ci % 2 == 0 else nc.sync
                r0 = t * C + ci * 128
                eng.dma_start(x_[:], xT[r0:r0 + 128, :])
                tiles.append(x_)
            return tiles

        # ---------------- t0: stream everything in ----------------
        # queue plan: sync=wq,wv-even,consts,cos/sin,(x-odd),wo
        #             scalar=x0,(x-even)   gpsimd=wk,wv-odd
        xt_cur = []
        for ci in range(NCH):
            wt = wpool.tile([128, DH], bf16, tag=f"wq{ci}")
            nc.sync.dma_start(wt[:], wq[ci * 128:(ci + 1) * 128, :])
            w_tiles[("wq", ci)] = wt
            x_ = xpool.tile([128, TCH], bf16, tag=f"x{ci}")
            nc.scalar.dma_start(x_[:], xT[ci * 128:(ci + 1) * 128, :])
            xt_cur.append(x_)
            wt = wpool.tile([128, DH], bf16, tag=f"wk{ci}")
            nc.gpsimd.dma_start(wt[:], wk[ci * 128:(ci + 1) * 128, :])
            w_tiles[("wk", ci)] = wt
        for ci in range(NCH):
            wt = wpool.tile([128, DH], bf16, tag=f"wv{ci}")
            eng = nc.sync if ci % 2 == 0 else nc.gpsimd
            eng.dma_start(wt[:], wv[ci * 128:(ci + 1) * 128, :])
            w_tiles[("wv", ci)] = wt
        # rope tables on the gpsimd queue after wv (V(0) was stalling on
        # late wv chunks when 1MB of tables sat between wk and wv; first
        # rope doesn't need them until ~31us)
        nc.gpsimd.dma_start(cs2_t[:], cos2)
        nc.gpsimd.dma_start(sn2_t[:], sin2)
        nc.sync.dma_start(mb_t[:], mbd)
        _emit_qk_proj_t0(xt_cur)
        xt_next = _prefetch_x(1)
        _emit_v_proj_t0(xt_cur)
        for j in range(HPC):
            for cc in range(C // 512):
                wt_ = wopool.tile([128, 512], bf16, tag=f"wo{j}_{cc}")
                nc.sync.dma_start(
                    wt_[:], wo[j * 128:(j + 1) * 128, cc * 512:(cc + 1) * 512])
                wo_tiles[(j, cc)] = wt_

        # ------- steady pipeline: attn(t) + wo(t-1) + proj(t+1) per head ---
        prev_ot = None
        xt_next2 = [None]
        for t in range(NT):
            if t >= 1:
                xt_cur = xt_next
                xt_next = xt_next2[0]
                xt_next2[0] = None
            qc = t
            kmax = 4 * qc + 3
            for h in range(HPC):
                q_sl = qT[(h, qc)]
                pso_t = pso.tile([128, TCH], f32, tag="o")
                psr_t = psr.tile([128, TCH], f32, tag="rs")
                av_q = []       # pending blocks, popped in same-bank pairs
                quads = {}      # quad idx -> bf16 DVE-summed e tile
                rs_state = {"first": True}

                def _emit_rs(src_ap, qlo, last, rs_state=rs_state, psr_t=psr_t):
                    nc.tensor.matmul(
                        psr_t[0:1, qlo:], onesk, src_ap,
                        start=rs_state["first"], stop=last,
                    )
                    rs_state["first"] = False

                def _emit_av_pair(h=h, pso_t=pso_t, kmax=kmax, qc=qc,
                                  quads=quads):
                    pair = [av_q.pop(0), av_q.pop(0)]
                    for kb, qlo, e in pair:
                        nc.tensor.matmul(
                            pso_t[:, qlo:],
                            vb[kb][:, h * 128:(h + 1) * 128],
                            e[:, qlo:],
                            start=(kb == 0), stop=(kb == kmax),
                        )
                    for kb, qlo, e in pair:
                        if kb < 4 * qc:
                            # sub-diagonal: one rowsum matmul per summed quad
                            if kb % 4 == 3:
                                sq = quads.pop(kb // 4)
                                _emit_rs(sq[:], 0, False)
                        else:
                            _emit_rs(e[:, qlo:], qlo, kb == kmax)

                for kb in range(kmax + 1):
                    i_rel = kb - 4 * qc
                    qlo = 128 * i_rel if i_rel > 0 else 0
                    pss_t = pss.tile([128, TCH], f32, tag="s")
                    nc.tensor.matmul(
                        pss_t[:, qlo:],
                        kT[(h, kb // 4)][:, (kb % 4) * 128:(kb % 4 + 1) * 128],
                        q_sl[:, qlo:],
                        start=True,
                        stop=True,
                    )
                    e = ep.tile([128, TCH], bf16, tag="e")
                    nc.scalar.activation(
                        e[:, qlo:], pss_t[:, qlo:], AF.Exp, scale=SCALE
                    )
                    if i_rel >= 0:  # triangle mask on the diagonal square
                        nc.gpsimd.tensor_mul(
                            e[:, qlo:qlo + 128],
                            e[:, qlo:qlo + 128],
                            mb_t[:],
                        )
                    elif kb % 4 > 0:
                        # accumulate sub-diagonal quad rowsums on DVE (bf16)
                        g = kb // 4
                        if kb % 4 == 1:
                            sq = sqp.tile([128, TCH], bf16, tag="sq")
                            nc.vector.tensor_add(sq[:], prev_e[:], e[:])
                            quads[g] = sq
                        else:
                            nc.vector.tensor_add(quads[g][:], quads[g][:], e[:])
                    prev_e = e
                    av_q.append((kb, qlo, e))
                    if kb == 1 and pending[0] is not None:
                        _emit_norm(*pending[0])
                        pending[0] = None
                    if len(av_q) >= 6:
                        _emit_av_pair()
                if qc == 0 and t + 1 < NT:
                    # t=0 heads have only 4 blocks -- the exp/mask pipeline
                    # never fills. Sandwich the AV pairs between the Q and K
                    # projection chains so the PE doesn't expose the
                    # ACT/Pool latency.
                    _emit_one_proj(t + 1, h, "wq", qT, xt_next)
                    _emit_av_pair()
                    _emit_one_proj(t + 1, h, "wk", kT, xt_next)
                while av_q:
                    _emit_av_pair()
                pending[0] = (h, pso_t, psr_t)
                if qc >= 1:
                    _emit_wo_group(qc - 1, h, prev_ot)
                if t + 1 < NT:
                    if qc >= 1:
                        _emit_qk_head_proj(t + 1, h, xt_next)
                    # during chunk 0, ACT is backed up behind attention's
                    # exps -- route these copies to the idle DVE instead
                    _emit_v_tsi(t + 1, h, xt_next, use_act=(qc != 0))
                if h == 2 and t + 2 < NT:
                    # prefetch x(t+2) 1.5 heads before chunk t+1 starts; the
                    # x pool is triple-buffered so this aliases x(t-1), whose
                    # consumers are all emitted by now
                    xt_next2[0] = _prefetch_x(t + 2)
            _emit_norm(*pending[0])
            pending[0] = None
            prev_ot = dict(ot_tiles)
            ot_tiles = {}

        # ---------------- tail: wo for the last chunk ----------------
        for qs in range(TCH // 128):
            _emit_wo_group(NT - 1, qs, prev_ot, alt_copies=True)

    _split_waits(nc)
    return nc


_CACHED_NC = None


def _get_nc():
    global _CACHED_NC
    if _CACHED_NC is None:
        _CACHED_NC = _build_nc()
    return _CACHED_NC


# --------------------------------------------------------------------------
# host-side input prep / gather
# --------------------------------------------------------------------------
def _deinterleave_perm():
    """per-head column permutation: [2j for j<64] then [2j+1]"""
    p = np.empty(D, dtype=np.int64)
    p[:64] = np.arange(0, D, 2)
    p[64:] = np.arange(1, D, 2)
    return p


def _make_core_inputs(x, freqs_cos, freqs_sin, wq, wk, wv, wo):
    x = np.asarray(x, dtype=np.float32)
    freqs_cos = np.asarray(freqs_cos, dtype=np.float32)
    freqs_sin = np.asarray(freqs_sin, dtype=np.float32)
    wq = np.asarray(wq, dtype=np.float32)
    wk = np.asarray(wk, dtype=np.float32)
    wv = np.asarray(wv, dtype=np.float32)
    wo = np.asarray(wo, dtype=np.float32)

    perm = _deinterleave_perm()
    cosT = np.ascontiguousarray(freqs_cos.T)  # [64, T]
    sinT = np.ascontiguousarray(freqs_sin.T)
    cos2 = np.concatenate([cosT, cosT], axis=0).astype(BF16NP)  # [128, T]
    sin2 = np.concatenate([sinT, sinT], axis=0).astype(BF16NP)

    # causal triangle for the diagonal 128x128 square: mb[k, q] = 1 iff k <= q
    k_idx = np.arange(128)[:, None]
    q_idx = np.arange(128)[None, :]
    mb = (k_idx <= q_idx).astype(BF16NP)

    # x[b].T is [C, T]; restack as NT blocks of [C, TCH] so each (t, ci)
    # chunk is contiguous in DRAM.
    xTb = []
    for b in range(B):
        xt = x[b].T.reshape(C, NT, TCH).transpose(1, 0, 2).reshape(NT * C, TCH)
        xTb.append(np.ascontiguousarray(xt).astype(BF16NP))

    in_maps = []
    for core in range(N_CORES):
        b, hg = core // 4, core % 4
        cols = slice(hg * DH, (hg + 1) * DH)
        wq_s = wq[:, cols].reshape(C, HPC, D)[:, :, perm].reshape(C, DH)
        wk_s = wk[:, cols].reshape(C, HPC, D)[:, :, perm].reshape(C, DH)
        in_maps.append({
            "xT": xTb[b],
            "wq": np.ascontiguousarray(wq_s).astype(BF16NP),
            "wk": np.ascontiguousarray(wk_s).astype(BF16NP),
            "wv": np.ascontiguousarray(wv[:, cols]).astype(BF16NP),
            "wo": np.ascontiguousarray(wo[cols, :]).astype(BF16NP),
            "cos2": cos2,
            "sin2": sin2,
            "mb": mb,
        })
    return in_maps


def kernel(x, freqs_cos, freqs_sin, wq, wk, wv, wo, _trace=False, _trace_kwargs=None):
    nc = _get_nc()
    in_maps = _make_core_inputs(x, freqs_cos, freqs_sin, wq, wk, wv, wo)
    res = run_bass_kernel_spmd(
        nc, in_maps, core_ids=list(range(N_CORES)), trace=_trace,
        **(_trace_kwargs or {}),
    )
    out = np.zeros((B, T, C), dtype=np.float32)
    for core in range(N_CORES):
        # y is [4, T, 512] column-blocks of the [T, C] partial product
        yb = np.asarray(res.results[core]["y"], dtype=np.float32)
        yb = yb.reshape(4, T, 512).transpose(1, 0, 2).reshape(T, C)
        out[core // 4] += yb
    if _trace:
        kernel.last_results = res
    return out


# revision 34
# speedup vs baseline: 1.0795x; 1.0795x over previous
"""Multi-head attention (B=2, T=2048, C=2048, H=16, causal, rotary) on 8
Trainium2 NeuronCores.

Sharding: tensor-parallel over heads x data-parallel over batch.
Core c handles batch b = c // 4 and heads [4*(c%4), 4*(c%4)+4).
Each core computes a partial output y_c = attn_out(4 heads) @ wo_rows;
the host sums the 4 partials per batch (row-parallel wo).

v5 design. Measured facts this build encodes: back-to-back matmuls into
the SAME psum bank pipeline at ~1 col/cycle; every psum-bank switch
costs ~45ns; a start+stop matmul's visible latency includes a ~173ns
pipeline drain. So long same-bank accumulation runs are kept intact
(fine-grained interleaving of independent work into attention made
everything slower in v4), and attention cuts its matmul count instead:
  - rowsums for full (sub-diagonal) quads of key-blocks are pre-summed
    on DVE (bf16, 2x mode) and hit the PE as ONE ones-stationary matmul
    per quad; only the 4 diagonal blocks keep per-block rowsums.
  - AV (and diagonal rowsum) matmuls are emitted in adjacent pairs so
    every second one continues a same-bank accumulation chain.
  - wo(t-1) + proj(t+1) are emitted per head after attention(t,h)
    (head-granularity interleave).
  - rope: 3 DVE muls (partition-swapped sin products; PSUM operand is
    exempt from the equal-base-partition rule) + sub/add alternating
    DVE/GpSimd. t0 ropes run in (q_h,k_h) pairs so head h's tables are
    ready when attention(0,h) arrives.
  - x staged [NT*C, TCH] and y [4*T, 512]: all big DMAs contiguous.
  - psum banks: pp(proj+wo+norm bcast)=2, pss(scores)=3, pso(AV)=2,
    psr(rowsum)=1.
"""

import math
import os
import sys
from contextlib import ExitStack

import numpy as np

for _p in ("/opt/trn_rl_repo", "/root/.axon_site/_ro/trn_rl_repo"):
    if os.path.isdir(_p) and _p not in sys.path:
        sys.path.append(_p)

import bass_rust
import ml_dtypes
import concourse.bass as bass
import concourse.mybir as mybir
import concourse.tile as tile
from concourse import library_config
from concourse.bass_utils import run_bass_kernel_spmd
from concourse.vector_clock import ScopedClock, VectorClock

B, T, C, H = 2, 2048, 2048, 16
D = 128
HPC = H // 4          # 4 heads per core
DH = HPC * D          # 512 head-dims per core
NCH = C // 128        # 16 contraction chunks
TCH = 512             # t-chunk == q-chunk
NT = T // TCH         # 4
N_CORES = 8
SCALE = 1.0 / math.sqrt(D)

f32 = mybir.dt.float32
f32r = mybir.dt.float32r
bf16 = mybir.dt.bfloat16
AF = mybir.ActivationFunctionType
BF16NP = ml_dtypes.bfloat16


# --------------------------------------------------------------------------
# toolchain workarounds (from the known-good baseline)
# --------------------------------------------------------------------------
def _patched_drain_and_barrier(self, tick_clock, wait_clock):
    """walrus codegen accepts only one sem wait on an InstDrain; emit one
    drain per outstanding proc instead of one drain with N waits."""
    ticks = list(tick_clock.global_clock)
    for i, t in enumerate(ticks):
        if t <= 0:
            continue
        sub = VectorClock([t if j == i else 0 for j in range(len(ticks))])
        d = self.nc.sync.drain()
        wait_clock.add_sem_waits(d.ins, ScopedClock({None: sub}))
    self.nc.all_engine_barrier()
    assert self.sems is not None
    popped = self.nc._tile_sem_poison_stack.pop()
    assert popped is self._sem_poison
    self.nc.clear_and_free_semaphores(list(self.sems.allocated().values()))
    self.nc.all_engine_barrier()


tile.TileContext._drain_and_barrier = _patched_drain_and_barrier

_SPLIT_OPS = {
    "Matmult", "Drain", "DMACopy", "DMATransposeAnt", "Activation", "TensorTensor", "TensorReduce",
    "TensorCopy", "Reciprocal", "TensorScalarPtr", "TensorScalar",
    "CopyPredicated", "Memset", "NoOp", "Pool", "Max", "MaxIndex",
    "StreamShuffle", "StreamTranspose", "TensorTensorScan",
    "ScalarTensorTensor", "TensorTensorReduce", "Iota", "BNStats",
    "BNStatsAggregate", "Select", "PartitionBroadcast",
}
_ws_counter = [0]


def _split_waits(nc, limit=1):
    """walrus encodes a limited number of sem waits on engine instructions
    (fused bf16 LDW+MM and Drain take only one). Move excess waits onto
    same-engine NoOps inserted immediately before; engine program order
    preserves semantics."""
    for f in nc.m.functions:
        for b in f.blocks:
            insts = b.instructions
            i = 0
            while i < len(insts):
                inst = insts[i]
                si = inst.sync_info
                if (
                    inst.opcode not in _SPLIT_OPS
                    or si is None
                    or not si.on_wait
                    or len(si.on_wait) <= limit
                ):
                    i += 1
                    continue
                waits = list(si.on_wait)
                extra, keep = waits[:-limit], waits[-limit:]
                for w in extra:
                    _ws_counter[0] += 1
                    nop = bass_rust.InstNoOp(
                        name=f"I-waitsplit-{_ws_counter[0]}", engine=inst.engine
                    )
                    nop.sync_info = mybir.SyncInfo(on_wait=[w], on_update=[])
                    insts.insert(i, nop)
                    i += 1
                inst.sync_info = mybir.SyncInfo(
                    on_wait=keep,
                    on_update=list(si.on_update) if si.on_update else [],
                )
                i += 1


# --------------------------------------------------------------------------
# kernel build
# --------------------------------------------------------------------------
def _build_nc():
    nc = bass.Bass("TRN2", debug=False, target_bir_lowering=False)

    # x staged as NT row-blocks of [C, TCH] so each (t, ci) chunk is one
    # contiguous 128KB region; y as 4 column-blocks of [T, 512] likewise.
    xT = nc.dram_tensor("xT", [NT * C, TCH], bf16, kind="ExternalInput").ap()
    wq = nc.dram_tensor("wq", [C, DH], bf16, kind="ExternalInput").ap()
    wk = nc.dram_tensor("wk", [C, DH], bf16, kind="ExternalInput").ap()
    wv = nc.dram_tensor("wv", [C, DH], bf16, kind="ExternalInput").ap()
    wo = nc.dram_tensor("wo", [DH, C], bf16, kind="ExternalInput").ap()
    cos2 = nc.dram_tensor("cos2", [128, T], bf16, kind="ExternalInput").ap()
    sin2 = nc.dram_tensor("sin2", [128, T], bf16, kind="ExternalInput").ap()
    mbd = nc.dram_tensor("mb", [128, 128], bf16, kind="ExternalInput").ap()
    y = nc.dram_tensor("y", [4 * T, 512], bf16, kind="ExternalOutput").ap()

    with tile.TileContext(nc) as tc, ExitStack() as es:
        # ---- pools (whole-kernel lifetime) ----
        wpool = es.enter_context(tc.tile_pool(name="w", bufs=1))
        wopool = es.enter_context(tc.tile_pool(name="wo", bufs=1))
        xpool = es.enter_context(tc.tile_pool(name="x", bufs=3))
        cpool = es.enter_context(tc.tile_pool(name="cs", bufs=1))
        persist = es.enter_context(tc.tile_pool(name="qkv", bufs=1))
        rt = es.enter_context(tc.tile_pool(name="rt", bufs=2))
        ep = es.enter_context(tc.tile_pool(name="e", bufs=8))
        sqp = es.enter_context(tc.tile_pool(name="sq", bufs=2))
        otp = es.enter_context(tc.tile_pool(name="ot", bufs=2))
        rp = es.enter_context(tc.tile_pool(name="r", bufs=2))
        ysbp = es.enter_context(tc.tile_pool(name="ysb", bufs=4))

        pp = es.enter_context(tc.tile_pool(name="pp", bufs=2, space="PSUM"))
        pss = es.enter_context(tc.tile_pool(name="pss", bufs=3, space="PSUM"))
        pso = es.enter_context(tc.tile_pool(name="pso", bufs=2, space="PSUM"))
        psr = es.enter_context(tc.tile_pool(name="psr", bufs=1, space="PSUM"))

        # ---- persistent SBUF tensors ----
        # matmul operands are kept in [128, 512] tiles (1KB per-partition
        # rows): operands sliced from wider tiles measurably slow the PE
        # (~216ns vs ~259ns per 512-col matmul).
        qT = {(h, t): persist.tile([128, TCH], bf16, tag=f"qT{h}_{t}", name=f"qT{h}_{t}")
              for h in range(HPC) for t in range(NT)}
        kT = {(h, t): persist.tile([128, TCH], bf16, tag=f"kT{h}_{t}", name=f"kT{h}_{t}")
              for h in range(HPC) for t in range(NT)}
        # vb[kb][:, h*128:(h+1)*128] = V rows of key-block kb for head h
        vb = {kb: persist.tile([128, DH], bf16, tag=f"vb{kb}", name=f"vb{kb}")
              for kb in range(NT * 4)}

        w_tiles = {}
        cs2_t = cpool.tile([128, T], bf16, tag="cos")
        sn2_t = cpool.tile([128, T], bf16, tag="sin")
        mb_t = cpool.tile([128, 128], bf16, tag="mb")
        # all-ones views carved out of the mask tile (mb[:,127]==1 for all k,
        # mb[0,:]==1 for all q) -- saves two DMAs on the startup queue
        onesk = mb_t[:, 127:128]
        ones1 = mb_t[0:1, :]
        wo_tiles = {}

        ot_tiles = {}
        pending = [None]
        rope_par = [0]

        def _emit_norm(h, pso_t, psr_t):
            # 1/rowsum as exp(-ln(r)) on ACT over the tiny [1,512] rowsum
            # vector (DVE reciprocal() is a 3.2us multi-pass op and was
            # congesting the DVE queue), then broadcast across partitions
            # with a ones-stationary matmul and one DVE multiply.
            lnr = rp.tile([1, TCH], f32, tag="lnr")
            nc.scalar.activation(lnr[:], psr_t[0:1, :], AF.Ln)
            binv1 = rp.tile([1, TCH], bf16, tag="binv1")
            nc.scalar.activation(binv1[:], lnr[:], AF.Exp, scale=-1.0)
            nc.tensor.matmul(psr_t[:, :], ones1, binv1[:], start=True, stop=True)
            binv = rp.tile([128, TCH], bf16, tag="binv")
            nc.vector.tensor_copy(binv[:], psr_t[:, :])
            ot = otp.tile([128, TCH], bf16, tag=f"ot{h}")
            nc.vector.tensor_mul(ot[:], pso_t[:], binv[:])
            ot_tiles[h] = ot

        def _copy_out(dst, src, use_act):
            if use_act:
                nc.scalar.copy(dst, src)
            else:
                nc.vector.tensor_copy(dst, src)

        def _rope(ps, dst_t, tsl):
            # de-interleaved pairs: ps[0:64]=real, ps[64:128]=imag.
            # tA = ps*cos2 = [r*cos; i*cos]; tS = partition-swapped sin
            # products [i*sin; r*sin]. out_r = r*cos - i*sin,
            # out_i = r*sin + i*cos.
            tA = rt.tile([128, TCH], f32, tag="rA")
            nc.vector.tensor_mul(tA[:], ps[:], cs2_t[:, tsl])
            tS = rt.tile([128, TCH], f32, tag="rB")
            nc.vector.tensor_mul(tS[0:64, :], ps[64:128, :], sn2_t[0:64, tsl])
            nc.vector.tensor_mul(tS[64:128, :], ps[0:64, :], sn2_t[64:128, tsl])
            eng = nc.vector if rope_par[0] % 2 == 0 else nc.gpsimd
            rope_par[0] += 1
            eng.tensor_sub(dst_t[0:64, :], tA[0:64, :], tS[0:64, :])
            eng.tensor_add(dst_t[64:128, :], tS[64:128, :], tA[64:128, :])

        def _emit_wo_group(qc, qs, src_ot, alt_copies=False):
            # one q-subblock of the deferred output projection; interleaved
            # after each attention head
            row0 = qc * TCH + qs * 128
            for cc in range(C // 512):
                psy = pp.tile([128, 512], f32, tag="pp")
                for hh in range(HPC):
                    nc.tensor.matmul(
                        psy[:],
                        src_ot[hh][:, qs * 128:(qs + 1) * 128],
                        wo_tiles[(hh, cc)][:],
                        start=(hh == 0),
                        stop=(hh == HPC - 1),
                    )
                ysb_c = ysbp.tile([128, 512], bf16, tag="ysb")
                _copy_out(ysb_c[:], psy[:], alt_copies and cc % 2 == 1)
                nc.sync.dma_start(
                    y[cc * T + row0:cc * T + row0 + 128, :], ysb_c[:],
                )

        def _emit_one_proj(t, h, wname, dst, xt):
            tsl_ = bass.ts(t, TCH)
            ps = pp.tile([128, TCH], f32, tag="pp")
            for ci in range(NCH):
                nc.tensor.matmul(
                    ps[:],
                    w_tiles[(wname, ci)][:, h * 128:(h + 1) * 128],
                    xt[ci][:],
                    start=(ci == 0),
                    stop=(ci == NCH - 1),
                )
            _rope(ps, dst[(h, t)], tsl_)

        def _emit_qk_head_proj(t, h, xt):
            _emit_one_proj(t, h, "wq", qT, xt)
            _emit_one_proj(t, h, "wk", kT, xt)

        def _emit_v_tsi(t, tsi, xt):
            ps = pp.tile([128, DH], f32, tag="pp")
            for ci in range(NCH):
                nc.tensor.matmul(
                    ps[:],
                    xt[ci][:, tsi * 128:(tsi + 1) * 128],
                    w_tiles[("wv", ci)][:],
                    start=(ci == 0),
                    stop=(ci == NCH - 1),
                )
            kb = t * 4 + tsi
            _copy_out(vb[kb][:], ps[:], True)

        def _emit_qk_proj_t0(xt):
            # run 7 projection groups concurrently across the (still idle)
            # attention psum pools so the PE keeps pace with the w/x DMA
            # stream. Groups are ordered (q0,k0),(q1,k1),... and roped in
            # that order so head h's tables are ready when attention(0,h)
            # arrives. K-h3 is emitted interleaved with the wv-paced
            # V(0)-tsi0 group.
            tsl_ = bass.ts(0, TCH)
            _gp = [("wq", 0, pp, "pp"), ("wk", 0, pso, "o"),
                   ("wq", 1, pss, "s"), ("wk", 1, pso, "o"),
                   ("wq", 2, pss, "s"), ("wk", 2, pp, "pp"),
                   ("wq", 3, pss, "s")]
            groups = [
                (wn, h_, pool.tile([128, TCH], f32, tag=tg, name=f"g0_{wn}{h_}"))
                for wn, h_, pool, tg in _gp
            ]
            for ci in range(NCH):
                for wname, h, ps in groups:
                    nc.tensor.matmul(
                        ps[:],
                        w_tiles[(wname, ci)][:, h * 128:(h + 1) * 128],
                        xt[ci][:],
                        start=(ci == 0),
                        stop=(ci == NCH - 1),
                    )
            for gi in (0, 5, 1, 2, 3, 4, 6):
                wname, h, ps = groups[gi]
                _rope(ps, (qT if wname == "wq" else kT)[(h, 0)], tsl_)

        def _emit_v_proj_t0(xt):
            psk3 = pp.tile([128, TCH], f32, tag="pp", name="psk3")
            for tsi in range(TCH // 128):
                ps = pp.tile([128, DH], f32, tag="pp")
                for ci in range(NCH):
                    nc.tensor.matmul(
                        ps[:],
                        xt[ci][:, tsi * 128:(tsi + 1) * 128],
                        w_tiles[("wv", ci)][:],
                        start=(ci == 0),
                        stop=(ci == NCH - 1),
                    )
                    if tsi == 0:
                        nc.tensor.matmul(
                            psk3[:],
                            w_tiles[("wk", ci)][:, 3 * 128:4 * 128],
                            xt[ci][:],
                            start=(ci == 0), stop=(ci == NCH - 1),
                        )
                if tsi == 0:
                    _rope(psk3, kT[(3, 0)], bass.ts(0, TCH))
                _copy_out(vb[tsi][:], ps[:], True)

        def _prefetch_x(t):
            tiles = []
            for ci in range(NCH):
                x_ = xpool.tile([128, TCH], bf16, tag=f"x{ci}")
                eng = nc.scalar if ci % 2 == 0 else nc.sync
                r0 = t * C + ci * 128
                eng.dma_start(x_[:], xT[r0:r0 + 128, :])
                tiles.append(x_)
            return tiles

        # ---------------- t0: stream everything in ----------------
        # queue plan: sync=wq,wv-even,consts,cos/sin,(x-odd),wo
        #             scalar=x0,(x-even)   gpsimd=wk,wv-odd
        xt_cur = []
        for ci in range(NCH):
            wt = wpool.tile([128, DH], bf16, tag=f"wq{ci}")
            nc.sync.dma_start(wt[:], wq[ci * 128:(ci + 1) * 128, :])
            w_tiles[("wq", ci)] = wt
            x_ = xpool.tile([128, TCH], bf16, tag=f"x{ci}")
            nc.scalar.dma_start(x_[:], xT[ci * 128:(ci + 1) * 128, :])
            xt_cur.append(x_)
            wt = wpool.tile([128, DH], bf16, tag=f"wk{ci}")
            nc.gpsimd.dma_start(wt[:], wk[ci * 128:(ci + 1) * 128, :])
            w_tiles[("wk", ci)] = wt
        for ci in range(NCH):
            wt = wpool.tile([128, DH], bf16, tag=f"wv{ci}")
            eng = nc.sync if ci % 2 == 0 else nc.gpsimd
            eng.dma_start(wt[:], wv[ci * 128:(ci + 1) * 128, :])
            w_tiles[("wv", ci)] = wt
        # rope tables on the gpsimd queue after wv (V(0) was stalling on
        # late wv chunks when 1MB of tables sat between wk and wv; first
        # rope doesn't need them until ~31us)
        nc.gpsimd.dma_start(cs2_t[:], cos2)
        nc.gpsimd.dma_start(sn2_t[:], sin2)
        nc.sync.dma_start(mb_t[:], mbd)
        _emit_qk_proj_t0(xt_cur)
        xt_next = _prefetch_x(1)
        _emit_v_proj_t0(xt_cur)
        for j in range(HPC):
            for cc in range(C // 512):
                wt_ = wopool.tile([128, 512], bf16, tag=f"wo{j}_{cc}")
                nc.sync.dma_start(
                    wt_[:], wo[j * 128:(j + 1) * 128, cc * 512:(cc + 1) * 512])
                wo_tiles[(j, cc)] = wt_

        # ------- steady pipeline: attn(t) + wo(t-1) + proj(t+1) per head ---
        prev_ot = None
        xt_next2 = [None]
        for t in range(NT):
            if t >= 1:
                xt_cur = xt_next
                xt_next = xt_next2[0]
                xt_next2[0] = None
            qc = t
            kmax = 4 * qc + 3
            for h in range(HPC):
                q_sl = qT[(h, qc)]
                pso_t = pso.tile([128, TCH], f32, tag="o")
                psr_t = psr.tile([128, TCH], f32, tag="rs")
                av_q = []       # pending blocks, popped in same-bank pairs
                quads = {}      # quad idx -> bf16 DVE-summed e tile
                rs_state = {"first": True}

                def _emit_rs(src_ap, qlo, last, rs_state=rs_state, psr_t=psr_t):
                    nc.tensor.matmul(
                        psr_t[0:1, qlo:], onesk, src_ap,
                        start=rs_state["first"], stop=last,
                    )
                    rs_state["first"] = False

                def _emit_av_pair(h=h, pso_t=pso_t, kmax=kmax, qc=qc,
                                  quads=quads):
                    pair = [av_q.pop(0), av_q.pop(0)]
                    for kb, qlo, e in pair:
                        nc.tensor.matmul(
                            pso_t[:, qlo:],
                            vb[kb][:, h * 128:(h + 1) * 128],
                            e[:, qlo:],
                            start=(kb == 0), stop=(kb == kmax),
                        )
                    for kb, qlo, e in pair:
                        if kb < 4 * qc:
                            # sub-diagonal: one rowsum matmul per summed quad
                            if kb % 4 == 3:
                                sq = quads.pop(kb // 4)
                                _emit_rs(sq[:], 0, False)
                        else:
                            _emit_rs(e[:, qlo:], qlo, kb == kmax)

                for kb in range(kmax + 1):
                    i_rel = kb - 4 * qc
                    qlo = 128 * i_rel if i_rel > 0 else 0
                    pss_t = pss.tile([128, TCH], f32, tag="s")
                    nc.tensor.matmul(
                        pss_t[:, qlo:],
                        kT[(h, kb // 4)][:, (kb % 4) * 128:(kb % 4 + 1) * 128],
                        q_sl[:, qlo:],
                        start=True,
                        stop=True,
                    )
                    e = ep.tile([128, TCH], bf16, tag="e")
                    nc.scalar.activation(
                        e[:, qlo:], pss_t[:, qlo:], AF.Exp, scale=SCALE
                    )
                    if i_rel >= 0:  # triangle mask on the diagonal square
                        nc.gpsimd.tensor_mul(
                            e[:, qlo:qlo + 128],
                            e[:, qlo:qlo + 128],
                            mb_t[:],
                        )
                    elif kb % 4 > 0:
                        # accumulate sub-diagonal quad rowsums on DVE (bf16)
                        g = kb // 4
                        if kb % 4 == 1:
                            sq = sqp.tile([128, TCH], bf16, tag="sq")
                            nc.vector.tensor_add(sq[:], prev_e[:], e[:])
                            quads[g] = sq
                        else:
                            nc.vector.tensor_add(quads[g][:], quads[g][:], e[:])
                    prev_e = e
                    av_q.append((kb, qlo, e))
                    if kb == 1 and pending[0] is not None:
                        _emit_norm(*pending[0])
                        pending[0] = None
                    if len(av_q) >= 6:
                        _emit_av_pair()
                if qc == 0 and t + 1 < NT:
                    # t=0 heads have only 4 blocks -- the exp/mask pipeline
                    # never fills. Sandwich the AV pairs between the Q and K
                    # projection chains so the PE doesn't expose the
                    # ACT/Pool latency.
                    _emit_one_proj(t + 1, h, "wq", qT, xt_next)
                    _emit_av_pair()
                    _emit_one_proj(t + 1, h, "wk", kT, xt_next)
                while av_q:
                    _emit_av_pair()
                pending[0] = (h, pso_t, psr_t)
                if qc >= 1:
                    _emit_wo_group(qc - 1, h, prev_ot)
                if t + 1 < NT:
                    if qc >= 1:
                        _emit_qk_head_proj(t + 1, h, xt_next)
                    _emit_v_tsi(t + 1, h, xt_next)
                if h == 2 and t + 2 < NT:
                    # prefetch x(t+2) 1.5 heads before chunk t+1 starts; the
                    # x pool is triple-buffered so this aliases x(t-1), whose
                    # consumers are all emitted by now
                    xt_next2[0] = _prefetch_x(t + 2)
            _emit_norm(*pending[0])
            pending[0] = None
            prev_ot = dict(ot_tiles)
            ot_tiles = {}

        # ---------------- tail: wo for the last chunk ----------------
        for qs in range(TCH // 128):
            _emit_wo_group(NT - 1, qs, prev_ot, alt_copies=True)

    _split_waits(nc)
    return nc


_CACHED_NC = None


def _get_nc():
    global _CACHED_NC
    if _CACHED_NC is None:
        _CACHED_NC = _build_nc()
    return _CACHED_NC


# --------------------------------------------------------------------------
# host-side input prep / gather
# --------------------------------------------------------------------------
def _deinterleave_perm():
    """per-head column permutation: [2j for j<64] then [2j+1]"""
    p = np.empty(D, dtype=np.int64)
    p[:64] = np.arange(0, D, 2)
    p[64:] = np.arange(1, D, 2)
    return p


def _make_core_inputs(x, freqs_cos, freqs_sin, wq, wk, wv, wo):
    x = np.asarray(x, dtype=np.float32)
    freqs_cos = np.asarray(freqs_cos, dtype=np.float32)
    freqs_sin = np.asarray(freqs_sin, dtype=np.float32)
    wq = np.asarray(wq, dtype=np.float32)
    wk = np.asarray(wk, dtype=np.float32)
    wv = np.asarray(wv, dtype=np.float32)
    wo = np.asarray(wo, dtype=np.float32)

    perm = _deinterleave_perm()
    cosT = np.ascontiguousarray(freqs_cos.T)  # [64, T]
    sinT = np.ascontiguousarray(freqs_sin.T)
    cos2 = np.concatenate([cosT, cosT], axis=0).astype(BF16NP)  # [128, T]
    sin2 = np.concatenate([sinT, sinT], axis=0).astype(BF16NP)

    # causal triangle for the diagonal 128x128 square: mb[k, q] = 1 iff k <= q
    k_idx = np.arange(128)[:, None]
    q_idx = np.arange(128)[None, :]
    mb = (k_idx <= q_idx).astype(BF16NP)

    # x[b].T is [C, T]; restack as NT blocks of [C, TCH] so each (t, ci)
    # chunk is contiguous in DRAM.
    xTb = []
    for b in range(B):
        xt = x[b].T.reshape(C, NT, TCH).transpose(1, 0, 2).reshape(NT * C, TCH)
        xTb.append(np.ascontiguousarray(xt).astype(BF16NP))

    in_maps = []
    for core in range(N_CORES):
        b, hg = core // 4, core % 4
        cols = slice(hg * DH, (hg + 1) * DH)
        wq_s = wq[:, cols].reshape(C, HPC, D)[:, :, perm].reshape(C, DH)
        wk_s = wk[:, cols].reshape(C, HPC, D)[:, :, perm].reshape(C, DH)
        in_maps.append({
            "xT": xTb[b],
            "wq": np.ascontiguousarray(wq_s).astype(BF16NP),
            "wk": np.ascontiguousarray(wk_s).astype(BF16NP),
            "wv": np.ascontiguousarray(wv[:, cols]).astype(BF16NP),
            "wo": np.ascontiguousarray(wo[cols, :]).astype(BF16NP),
            "cos2": cos2,
            "sin2": sin2,
            "mb": mb,
        })
    return in_maps


def kernel(x, freqs_cos, freqs_sin, wq, wk, wv, wo, _trace=False, _trace_kwargs=None):
    nc = _get_nc()
    in_maps = _make_core_inputs(x, freqs_cos, freqs_sin, wq, wk, wv, wo)
    res = run_bass_kernel_spmd(
        nc, in_maps, core_ids=list(range(N_CORES)), trace=_trace,
        **(_trace_kwargs or {}),
    )
    out = np.zeros((B, T, C), dtype=np.float32)
    for core in range(N_CORES):
        # y is [4, T, 512] column-blocks of the [T, C] partial product
        yb = np.asarray(res.results[core]["y"], dtype=np.float32)
        yb = yb.reshape(4, T, 512).transpose(1, 0, 2).reshape(T, C)
        out[core // 4] += yb
    if _trace:
        kernel.last_results = res
    return out


# revision 36
# speedup vs baseline: 1.0972x; 1.0164x over previous
"""Multi-head attention (B=2, T=2048, C=2048, H=16, causal, rotary) on 8
Trainium2 NeuronCores.

Sharding: tensor-parallel over heads x data-parallel over batch.
Core c handles batch b = c // 4 and heads [4*(c%4), 4*(c%4)+4).
Each core computes a partial output y_c = attn_out(4 heads) @ wo_rows;
the host sums the 4 partials per batch (row-parallel wo).

v13 design (final). Measured facts this build encodes: matmul operand
tiles must be [128,512] (1KB per-partition rows; wider-tile slices cost
~20%/matmul); back-to-back matmuls into
the SAME psum bank pipeline at ~1 col/cycle; every psum-bank switch
costs ~45ns; a start+stop matmul's visible latency includes a ~173ns
pipeline drain. So long same-bank accumulation runs are kept intact
(fine-grained interleaving of independent work into attention made
everything slower in v4), and attention cuts its matmul count instead:
  - rowsums for full (sub-diagonal) quads of key-blocks are pre-summed
    on DVE (bf16, 2x mode) and hit the PE as ONE ones-stationary matmul
    per quad; only the 4 diagonal blocks keep per-block rowsums.
  - AV (and diagonal rowsum) matmuls are emitted in adjacent pairs so
    every second one continues a same-bank accumulation chain.
  - wo(t-1) + proj(t+1) are emitted per head after attention(t,h)
    (head-granularity interleave).
  - rope: 3 DVE muls (partition-swapped sin products; PSUM operand is
    exempt from the equal-base-partition rule) + sub/add alternating
    DVE/GpSimd. t0 ropes run in (q_h,k_h) pairs so head h's tables are
    ready when attention(0,h) arrives.
  - x staged [NT*C, TCH] and y [4*T, 512]: all big DMAs contiguous.
  - 1/rowsum = exp(-ln(r)) on ACT over [1,512] (DVE reciprocal is a
    3.2us multi-pass op); the broadcast matmul reuses the rowsum's own
    psum bank (row 0 holds r, the ones-matmul overwrites in place).
  - psum banks: pp(proj+wo)=2, pss(scores)=3, pso(AV)=2,
    psr(rowsum+norm bcast)=1.
"""

import math
import os
import sys
from contextlib import ExitStack

import numpy as np

for _p in ("/opt/trn_rl_repo", "/root/.axon_site/_ro/trn_rl_repo"):
    if os.path.isdir(_p) and _p not in sys.path:
        sys.path.append(_p)

import bass_rust
import ml_dtypes
import concourse.bass as bass
import concourse.mybir as mybir
import concourse.tile as tile
from concourse import library_config
from concourse.bass_utils import run_bass_kernel_spmd
from concourse.vector_clock import ScopedClock, VectorClock

B, T, C, H = 2, 2048, 2048, 16
D = 128
HPC = H // 4          # 4 heads per core
DH = HPC * D          # 512 head-dims per core
NCH = C // 128        # 16 contraction chunks
TCH = 512             # t-chunk == q-chunk
NT = T // TCH         # 4
N_CORES = 8
SCALE = 1.0 / math.sqrt(D)

f32 = mybir.dt.float32
f32r = mybir.dt.float32r
bf16 = mybir.dt.bfloat16
AF = mybir.ActivationFunctionType
BF16NP = ml_dtypes.bfloat16


# --------------------------------------------------------------------------
# toolchain workarounds (from the known-good baseline)
# --------------------------------------------------------------------------
def _patched_drain_and_barrier(self, tick_clock, wait_clock):
    """walrus codegen accepts only one sem wait on an InstDrain; emit one
    drain per outstanding proc instead of one drain with N waits."""
    ticks = list(tick_clock.global_clock)
    for i, t in enumerate(ticks):
        if t <= 0:
            continue
        sub = VectorClock([t if j == i else 0 for j in range(len(ticks))])
        d = self.nc.sync.drain()
        wait_clock.add_sem_waits(d.ins, ScopedClock({None: sub}))
    self.nc.all_engine_barrier()
    assert self.sems is not None
    popped = self.nc._tile_sem_poison_stack.pop()
    assert popped is self._sem_poison
    self.nc.clear_and_free_semaphores(list(self.sems.allocated().values()))
    self.nc.all_engine_barrier()


tile.TileContext._drain_and_barrier = _patched_drain_and_barrier

_SPLIT_OPS = {
    "Matmult", "Drain", "DMACopy", "DMATransposeAnt", "Activation", "TensorTensor", "TensorReduce",
    "TensorCopy", "Reciprocal", "TensorScalarPtr", "TensorScalar",
    "CopyPredicated", "Memset", "NoOp", "Pool", "Max", "MaxIndex",
    "StreamShuffle", "StreamTranspose", "TensorTensorScan",
    "ScalarTensorTensor", "TensorTensorReduce", "Iota", "BNStats",
    "BNStatsAggregate", "Select", "PartitionBroadcast",
}
_ws_counter = [0]


def _split_waits(nc, limit=1):
    """walrus encodes a limited number of sem waits on engine instructions
    (fused bf16 LDW+MM and Drain take only one). Move excess waits onto
    same-engine NoOps inserted immediately before; engine program order
    preserves semantics."""
    for f in nc.m.functions:
        for b in f.blocks:
            insts = b.instructions
            i = 0
            while i < len(insts):
                inst = insts[i]
                si = inst.sync_info
                if (
                    inst.opcode not in _SPLIT_OPS
                    or si is None
                    or not si.on_wait
                    or len(si.on_wait) <= limit
                ):
                    i += 1
                    continue
                waits = list(si.on_wait)
                extra, keep = waits[:-limit], waits[-limit:]
                for w in extra:
                    _ws_counter[0] += 1
                    nop = bass_rust.InstNoOp(
                        name=f"I-waitsplit-{_ws_counter[0]}", engine=inst.engine
                    )
                    nop.sync_info = mybir.SyncInfo(on_wait=[w], on_update=[])
                    insts.insert(i, nop)
                    i += 1
                inst.sync_info = mybir.SyncInfo(
                    on_wait=keep,
                    on_update=list(si.on_update) if si.on_update else [],
                )
                i += 1


# --------------------------------------------------------------------------
# kernel build
# --------------------------------------------------------------------------
def _build_nc():
    nc = bass.Bass("TRN2", debug=False, target_bir_lowering=False)

    # x staged as NT row-blocks of [C, TCH] so each (t, ci) chunk is one
    # contiguous 128KB region; y as 4 column-blocks of [T, 512] likewise.
    xT = nc.dram_tensor("xT", [NT * C, TCH], bf16, kind="ExternalInput").ap()
    wq = nc.dram_tensor("wq", [C, DH], bf16, kind="ExternalInput").ap()
    wk = nc.dram_tensor("wk", [C, DH], bf16, kind="ExternalInput").ap()
    wv = nc.dram_tensor("wv", [C, DH], bf16, kind="ExternalInput").ap()
    wo = nc.dram_tensor("wo", [DH, C], bf16, kind="ExternalInput").ap()
    cos2 = nc.dram_tensor("cos2", [128, T], bf16, kind="ExternalInput").ap()
    sin2 = nc.dram_tensor("sin2", [128, T], bf16, kind="ExternalInput").ap()
    mbd = nc.dram_tensor("mb", [128, 128], bf16, kind="ExternalInput").ap()
    y = nc.dram_tensor("y", [4 * T, 512], bf16, kind="ExternalOutput").ap()

    with tile.TileContext(nc) as tc, ExitStack() as es:
        # ---- pools (whole-kernel lifetime) ----
        wpool = es.enter_context(tc.tile_pool(name="w", bufs=1))
        wopool = es.enter_context(tc.tile_pool(name="wo", bufs=1))
        xpool = es.enter_context(tc.tile_pool(name="x", bufs=3))
        cpool = es.enter_context(tc.tile_pool(name="cs", bufs=1))
        persist = es.enter_context(tc.tile_pool(name="qkv", bufs=1))
        rt = es.enter_context(tc.tile_pool(name="rt", bufs=2))
        ep = es.enter_context(tc.tile_pool(name="e", bufs=8))
        sqp = es.enter_context(tc.tile_pool(name="sq", bufs=2))
        otp = es.enter_context(tc.tile_pool(name="ot", bufs=2))
        rp = es.enter_context(tc.tile_pool(name="r", bufs=2))
        ysbp = es.enter_context(tc.tile_pool(name="ysb", bufs=4))

        pp = es.enter_context(tc.tile_pool(name="pp", bufs=2, space="PSUM"))
        pss = es.enter_context(tc.tile_pool(name="pss", bufs=3, space="PSUM"))
        pso = es.enter_context(tc.tile_pool(name="pso", bufs=2, space="PSUM"))
        psr = es.enter_context(tc.tile_pool(name="psr", bufs=1, space="PSUM"))

        # ---- persistent SBUF tensors ----
        # matmul operands are kept in [128, 512] tiles (1KB per-partition
        # rows): operands sliced from wider tiles measurably slow the PE
        # (~216ns vs ~259ns per 512-col matmul).
        qT = {(h, t): persist.tile([128, TCH], bf16, tag=f"qT{h}_{t}", name=f"qT{h}_{t}")
              for h in range(HPC) for t in range(NT)}
        kT = {(h, t): persist.tile([128, TCH], bf16, tag=f"kT{h}_{t}", name=f"kT{h}_{t}")
              for h in range(HPC) for t in range(NT)}
        # vb[kb][:, h*128:(h+1)*128] = V rows of key-block kb for head h
        vb = {kb: persist.tile([128, DH], bf16, tag=f"vb{kb}", name=f"vb{kb}")
              for kb in range(NT * 4)}

        w_tiles = {}
        cs2_t = cpool.tile([128, T], bf16, tag="cos")
        sn2_t = cpool.tile([128, T], bf16, tag="sin")
        mb_t = cpool.tile([128, 128], bf16, tag="mb")
        # all-ones views carved out of the mask tile (mb[:,127]==1 for all k,
        # mb[0,:]==1 for all q) -- saves two DMAs on the startup queue
        onesk = mb_t[:, 127:128]
        ones1 = mb_t[0:1, :]
        wo_tiles = {}

        ot_tiles = {}
        pending = [None]
        rope_par = [0]

        def _emit_norm(h, pso_t, psr_t):
            # 1/rowsum as exp(-ln(r)) on ACT over the tiny [1,512] rowsum
            # vector (DVE reciprocal() is a 3.2us multi-pass op and was
            # congesting the DVE queue), then broadcast across partitions
            # with a ones-stationary matmul and one DVE multiply.
            lnr = rp.tile([1, TCH], f32, tag="lnr")
            nc.scalar.activation(lnr[:], psr_t[0:1, :], AF.Ln)
            binv1 = rp.tile([1, TCH], bf16, tag="binv1")
            nc.scalar.activation(binv1[:], lnr[:], AF.Exp, scale=-1.0)
            nc.tensor.matmul(psr_t[:, :], ones1, binv1[:], start=True, stop=True)
            binv = rp.tile([128, TCH], bf16, tag="binv")
            nc.vector.tensor_copy(binv[:], psr_t[:, :])
            ot = otp.tile([128, TCH], bf16, tag=f"ot{h}")
            nc.vector.tensor_mul(ot[:], pso_t[:], binv[:])
            ot_tiles[h] = ot

        def _copy_out(dst, src, use_act):
            if use_act:
                nc.scalar.copy(dst, src)
            else:
                nc.vector.tensor_copy(dst, src)

        def _rope(ps, dst_t, tsl):
            # de-interleaved pairs: ps[0:64]=real, ps[64:128]=imag.
            # tA = ps*cos2 = [r*cos; i*cos]; tS = partition-swapped sin
            # products [i*sin; r*sin]. out_r = r*cos - i*sin,
            # out_i = r*sin + i*cos.
            tA = rt.tile([128, TCH], f32, tag="rA")
            nc.vector.tensor_mul(tA[:], ps[:], cs2_t[:, tsl])
            tS = rt.tile([128, TCH], f32, tag="rB")
            nc.vector.tensor_mul(tS[0:64, :], ps[64:128, :], sn2_t[0:64, tsl])
            nc.vector.tensor_mul(tS[64:128, :], ps[0:64, :], sn2_t[64:128, tsl])
            eng = nc.vector if rope_par[0] % 2 == 0 else nc.gpsimd
            rope_par[0] += 1
            eng.tensor_sub(dst_t[0:64, :], tA[0:64, :], tS[0:64, :])
            eng.tensor_add(dst_t[64:128, :], tS[64:128, :], tA[64:128, :])

        def _emit_wo_group(qc, qs, src_ot, alt_copies=False):
            # one q-subblock of the deferred output projection; interleaved
            # after each attention head
            row0 = qc * TCH + qs * 128
            for cc in range(C // 512):
                psy = pp.tile([128, 512], f32, tag="pp")
                for hh in range(HPC):
                    nc.tensor.matmul(
                        psy[:],
                        src_ot[hh][:, qs * 128:(qs + 1) * 128],
                        wo_tiles[(hh, cc)][:],
                        start=(hh == 0),
                        stop=(hh == HPC - 1),
                    )
                ysb_c = ysbp.tile([128, 512], bf16, tag="ysb")
                _copy_out(ysb_c[:], psy[:], alt_copies and cc % 2 == 1)
                nc.sync.dma_start(
                    y[cc * T + row0:cc * T + row0 + 128, :], ysb_c[:],
                )

        def _emit_one_proj(t, h, wname, dst, xt):
            tsl_ = bass.ts(t, TCH)
            ps = pp.tile([128, TCH], f32, tag="pp")
            for ci in range(NCH):
                nc.tensor.matmul(
                    ps[:],
                    w_tiles[(wname, ci)][:, h * 128:(h + 1) * 128],
                    xt[ci][:],
                    start=(ci == 0),
                    stop=(ci == NCH - 1),
                )
            _rope(ps, dst[(h, t)], tsl_)

        def _emit_qk_head_proj(t, h, xt):
            _emit_one_proj(t, h, "wq", qT, xt)
            _emit_one_proj(t, h, "wk", kT, xt)

        def _emit_v_tsi(t, tsi, xt):
            ps = pp.tile([128, DH], f32, tag="pp")
            for ci in range(NCH):
                nc.tensor.matmul(
                    ps[:],
                    xt[ci][:, tsi * 128:(tsi + 1) * 128],
                    w_tiles[("wv", ci)][:],
                    start=(ci == 0),
                    stop=(ci == NCH - 1),
                )
            kb = t * 4 + tsi
            _copy_out(vb[kb][:], ps[:], True)

        def _emit_qk_proj_t0(xt):
            # run 7 projection groups concurrently across the (still idle)
            # attention psum pools so the PE keeps pace with the w/x DMA
            # stream. Groups are ordered (q0,k0),(q1,k1),... and roped in
            # that order so head h's tables are ready when attention(0,h)
            # arrives. K-h3 is emitted interleaved with the wv-paced
            # V(0)-tsi0 group.
            tsl_ = bass.ts(0, TCH)
            _gp = [("wq", 0, pp, "pp"), ("wk", 0, pso, "o"),
                   ("wq", 1, pss, "s"), ("wk", 1, pso, "o"),
                   ("wq", 2, pss, "s"), ("wk", 2, pp, "pp"),
                   ("wq", 3, pss, "s")]
            groups = [
                (wn, h_, pool.tile([128, TCH], f32, tag=tg, name=f"g0_{wn}{h_}"))
                for wn, h_, pool, tg in _gp
            ]
            for ci in range(NCH):
                for wname, h, ps in groups:
                    nc.tensor.matmul(
                        ps[:],
                        w_tiles[(wname, ci)][:, h * 128:(h + 1) * 128],
                        xt[ci][:],
                        start=(ci == 0),
                        stop=(ci == NCH - 1),
                    )
            for gi in (0, 5, 1, 2, 3, 4, 6):
                wname, h, ps = groups[gi]
                _rope(ps, (qT if wname == "wq" else kT)[(h, 0)], tsl_)

        def _emit_v_proj_t0(xt):
            psk3 = pp.tile([128, TCH], f32, tag="pp", name="psk3")
            for tsi in range(TCH // 128):
                ps = pp.tile([128, DH], f32, tag="pp")
                for ci in range(NCH):
                    nc.tensor.matmul(
                        ps[:],
                        xt[ci][:, tsi * 128:(tsi + 1) * 128],
                        w_tiles[("wv", ci)][:],
                        start=(ci == 0),
                        stop=(ci == NCH - 1),
                    )
                    if tsi == 0:
                        nc.tensor.matmul(
                            psk3[:],
                            w_tiles[("wk", ci)][:, 3 * 128:4 * 128],
                            xt[ci][:],
                            start=(ci == 0), stop=(ci == NCH - 1),
                        )
                if tsi == 0:
                    _rope(psk3, kT[(3, 0)], bass.ts(0, TCH))
                _copy_out(vb[tsi][:], ps[:], True)

        def _prefetch_x(t):
            tiles = []
            for ci in range(NCH):
                x_ = xpool.tile([128, TCH], bf16, tag=f"x{ci}")
                eng = nc.scalar if ci % 2 == 0 else nc.sync
                r0 = t * C + ci * 128
                eng.dma_start(x_[:], xT[r0:r0 + 128, :])
                tiles.append(x_)
            return tiles

        # ---------------- t0: stream everything in ----------------
        # queue plan: sync=wq,wv-even,consts,cos/sin,(x-odd),wo
        #             scalar=x0,(x-even)   gpsimd=wk,wv-odd
        xt_cur = []
        for ci in range(NCH):
            wt = wpool.tile([128, DH], bf16, tag=f"wq{ci}")
            nc.sync.dma_start(wt[:], wq[ci * 128:(ci + 1) * 128, :])
            w_tiles[("wq", ci)] = wt
            x_ = xpool.tile([128, TCH], bf16, tag=f"x{ci}")
            nc.scalar.dma_start(x_[:], xT[ci * 128:(ci + 1) * 128, :])
            xt_cur.append(x_)
            wt = wpool.tile([128, DH], bf16, tag=f"wk{ci}")
            nc.gpsimd.dma_start(wt[:], wk[ci * 128:(ci + 1) * 128, :])
            w_tiles[("wk", ci)] = wt
        for ci in range(NCH):
            wt = wpool.tile([128, DH], bf16, tag=f"wv{ci}")
            eng = nc.sync if ci % 2 == 0 else nc.gpsimd
            eng.dma_start(wt[:], wv[ci * 128:(ci + 1) * 128, :])
            w_tiles[("wv", ci)] = wt
        # rope tables on the gpsimd queue after wv (V(0) was stalling on
        # late wv chunks when 1MB of tables sat between wk and wv; first
        # rope doesn't need them until ~31us)
        nc.gpsimd.dma_start(cs2_t[:], cos2)
        nc.gpsimd.dma_start(sn2_t[:], sin2)
        nc.sync.dma_start(mb_t[:], mbd)
        _emit_qk_proj_t0(xt_cur)
        xt_next = _prefetch_x(1)
        _emit_v_proj_t0(xt_cur)
        for j in range(HPC):
            for cc in range(C // 512):
                wt_ = wopool.tile([128, 512], bf16, tag=f"wo{j}_{cc}")
                nc.sync.dma_start(
                    wt_[:], wo[j * 128:(j + 1) * 128, cc * 512:(cc + 1) * 512])
                wo_tiles[(j, cc)] = wt_

        # ------- steady pipeline: attn(t) + wo(t-1) + proj(t+1) per head ---
        prev_ot = None
        xt_next2 = [None]
        for t in range(NT):
            if t >= 1:
                xt_cur = xt_next
                xt_next = xt_next2[0]
                xt_next2[0] = None
            qc = t
            kmax = 4 * qc + 3
            for h in range(HPC):
                q_sl = qT[(h, qc)]
                pso_t = pso.tile([128, TCH], f32, tag="o")
                psr_t = psr.tile([128, TCH], f32, tag="rs")
                av_q = []       # pending blocks, popped in same-bank pairs
                quads = {}      # quad idx -> bf16 DVE-summed e tile
                rs_state = {"first": True}

                def _emit_rs(src_ap, qlo, last, rs_state=rs_state, psr_t=psr_t):
                    nc.tensor.matmul(
                        psr_t[0:1, qlo:], onesk, src_ap,
                        start=rs_state["first"], stop=last,
                    )
                    rs_state["first"] = False

                def _emit_av_pair(h=h, pso_t=pso_t, kmax=kmax, qc=qc,
                                  quads=quads):
                    pair = [av_q.pop(0), av_q.pop(0)]
                    for kb, qlo, e in pair:
                        nc.tensor.matmul(
                            pso_t[:, qlo:],
                            vb[kb][:, h * 128:(h + 1) * 128],
                            e[:, qlo:],
                            start=(kb == 0), stop=(kb == kmax),
                        )
                    for kb, qlo, e in pair:
                        if kb < 4 * qc:
                            # sub-diagonal: one rowsum matmul per summed quad
                            if kb % 4 == 3:
                                sq = quads.pop(kb // 4)
                                _emit_rs(sq[:], 0, False)
                        else:
                            _emit_rs(e[:, qlo:], qlo, kb == kmax)

                for kb in range(kmax + 1):
                    i_rel = kb - 4 * qc
                    qlo = 128 * i_rel if i_rel > 0 else 0
                    pss_t = pss.tile([128, TCH], f32, tag="s")
                    nc.tensor.matmul(
                        pss_t[:, qlo:],
                        kT[(h, kb // 4)][:, (kb % 4) * 128:(kb % 4 + 1) * 128],
                        q_sl[:, qlo:],
                        start=True,
                        stop=True,
                    )
                    e = ep.tile([128, TCH], bf16, tag="e")
                    nc.scalar.activation(
                        e[:, qlo:], pss_t[:, qlo:], AF.Exp, scale=SCALE
                    )
                    if i_rel >= 0:  # triangle mask on the diagonal square
                        nc.gpsimd.tensor_mul(
                            e[:, qlo:qlo + 128],
                            e[:, qlo:qlo + 128],
                            mb_t[:],
                        )
                    elif kb % 4 > 0:
                        # accumulate sub-diagonal quad rowsums on DVE (bf16)
                        g = kb // 4
                        if kb % 4 == 1:
                            sq = sqp.tile([128, TCH], bf16, tag="sq")
                            nc.vector.tensor_add(sq[:], prev_e[:], e[:])
                            quads[g] = sq
                        else:
                            nc.vector.tensor_add(quads[g][:], quads[g][:], e[:])
                    prev_e = e
                    av_q.append((kb, qlo, e))
                    if kb == 1 and pending[0] is not None:
                        _emit_norm(*pending[0])
                        pending[0] = None
                    if len(av_q) >= 6:
                        _emit_av_pair()
                if qc == 0 and t + 1 < NT:
                    # t=0 heads have only 4 blocks -- the exp/mask pipeline
                    # never fills. Sandwich the AV pairs between the Q and K
                    # projection chains so the PE doesn't expose the
                    # ACT/Pool latency.
                    _emit_one_proj(t + 1, h, "wq", qT, xt_next)
                    _emit_av_pair()
                    _emit_one_proj(t + 1, h, "wk", kT, xt_next)
                while av_q:
                    _emit_av_pair()
                pending[0] = (h, pso_t, psr_t)
                if qc >= 1:
                    _emit_wo_group(qc - 1, h, prev_ot)
                if t + 1 < NT and qc >= 1:
                    _emit_qk_head_proj(t + 1, h, xt_next)
                    _emit_v_tsi(t + 1, h, xt_next)
                if h == 2 and t + 2 < NT:
                    # prefetch x(t+2) 1.5 heads before chunk t+1 starts; the
                    # x pool is triple-buffered so this aliases x(t-1), whose
                    # consumers are all emitted by now
                    xt_next2[0] = _prefetch_x(t + 2)
            _emit_norm(*pending[0])
            pending[0] = None
            if qc == 0 and t + 1 < NT:
                # chunk-0 V chains emitted together after the last norm:
                # per-head emission put their ACT copies behind attention's
                # exps and stalled projection starts through the pp ring;
                # here the copies queue after all chunk-0 exps, and the
                # solid matmul block covers the chunk 0->1 boundary
                for tsi_ in range(TCH // 128):
                    _emit_v_tsi(t + 1, tsi_, xt_next)
            prev_ot = dict(ot_tiles)
            ot_tiles = {}

        # ---------------- tail: wo for the last chunk ----------------
        for qs in range(TCH // 128):
            _emit_wo_group(NT - 1, qs, prev_ot, alt_copies=True)

    _split_waits(nc)
    return nc


_CACHED_NC = None


def _get_nc():
    global _CACHED_NC
    if _CACHED_NC is None:
        _CACHED_NC = _build_nc()
    return _CACHED_NC


# --------------------------------------------------------------------------
# host-side input prep / gather
# --------------------------------------------------------------------------
def _deinterleave_perm():
    """per-head column permutation: [2j for j<64] then [2j+1]"""
    p = np.empty(D, dtype=np.int64)
    p[:64] = np.arange(0, D, 2)
    p[64:] = np.arange(1, D, 2)
    return p


def _make_core_inputs(x, freqs_cos, freqs_sin, wq, wk, wv, wo):
    x = np.asarray(x, dtype=np.float32)
    freqs_cos = np.asarray(freqs_cos, dtype=np.float32)
    freqs_sin = np.asarray(freqs_sin, dtype=np.float32)
    wq = np.asarray(wq, dtype=np.float32)
    wk = np.asarray(wk, dtype=np.float32)
    wv = np.asarray(wv, dtype=np.float32)
    wo = np.asarray(wo, dtype=np.float32)

    perm = _deinterleave_perm()
    cosT = np.ascontiguousarray(freqs_cos.T)  # [64, T]
    sinT = np.ascontiguousarray(freqs_sin.T)
    cos2 = np.concatenate([cosT, cosT], axis=0).astype(BF16NP)  # [128, T]
    sin2 = np.concatenate([sinT, sinT], axis=0).astype(BF16NP)

    # causal triangle for the diagonal 128x128 square: mb[k, q] = 1 iff k <= q
    k_idx = np.arange(128)[:, None]
    q_idx = np.arange(128)[None, :]
    mb = (k_idx <= q_idx).astype(BF16NP)

    # x[b].T is [C, T]; restack as NT blocks of [C, TCH] so each (t, ci)
    # chunk is contiguous in DRAM.
    xTb = []
    for b in range(B):
        xt = x[b].T.reshape(C, NT, TCH).transpose(1, 0, 2).reshape(NT * C, TCH)
        xTb.append(np.ascontiguousarray(xt).astype(BF16NP))

    in_maps = []
    for core in range(N_CORES):
        b, hg = core // 4, core % 4
        cols = slice(hg * DH, (hg + 1) * DH)
        wq_s = wq[:, cols].reshape(C, HPC, D)[:, :, perm].reshape(C, DH)
        wk_s = wk[:, cols].reshape(C, HPC, D)[:, :, perm].reshape(C, DH)
        in_maps.append({
            "xT": xTb[b],
            "wq": np.ascontiguousarray(wq_s).astype(BF16NP),
            "wk": np.ascontiguousarray(wk_s).astype(BF16NP),
            "wv": np.ascontiguousarray(wv[:, cols]).astype(BF16NP),
            "wo": np.ascontiguousarray(wo[cols, :]).astype(BF16NP),
            "cos2": cos2,
            "sin2": sin2,
            "mb": mb,
        })
    return in_maps


def kernel(x, freqs_cos, freqs_sin, wq, wk, wv, wo, _trace=False, _trace_kwargs=None):
    nc = _get_nc()
    in_maps = _make_core_inputs(x, freqs_cos, freqs_sin, wq, wk, wv, wo)
    res = run_bass_kernel_spmd(
        nc, in_maps, core_ids=list(range(N_CORES)), trace=_trace,
        **(_trace_kwargs or {}),
    )
    out = np.zeros((B, T, C), dtype=np.float32)
    for core in range(N_CORES):
        # y is [4, T, 512] column-blocks of the [T, C] partial product
        yb = np.asarray(res.results[core]["y"], dtype=np.float32)
        yb = yb.reshape(4, T, 512).transpose(1, 0, 2).reshape(T, C)
        out[core // 4] += yb
    if _trace:
        kernel.last_results = res
    return out
